# revision 25
# baseline (speedup 1.0000x reference)
"""GCNConv forward on 8 Trainium2 NeuronCores.

out = D^{-1/2} @ A @ x @ W + bias,  A sparse (edge list), D = row-degree.

Strategy: shard destination rows across the 8 cores; edge lists bucketed
by destination row on the host; x replicated per core so each core
gathers its source rows from local HBM; the segment-sum rides the
TensorEngine as matmuls against one-hot matrices; the 128x128 weight is
applied as a second matmul.

Fast path (uniform degree K, unit edge values): x pre-cast to bf16 on
the host; per 128-dest tile the K*128 edges are SORTED BY SOURCE and cut
at fixed positions into 5 cells, each spanning < 32768 source rows so a
multi-index dma_gather (InstDMAGatherAnt, int16 indices against a
per-cell base window) fetches 128*CB rows per instruction — one DMA
descriptor per edge, amortizing the ~1us/instruction SWDGE fixed
overhead that bounds the naive per-slot indirect-DMA version. Because
the sort permutes edges, dest ids travel as data and the one-hot
aggregation matrices are built on-device with batched is_equal against
an iota constant. The uniform D^{-1/2} = 1/sqrt(K) is folded into W on
the host; matmuls run in bf16 with fp32 PSUM accumulation.

kernel() accepts the FULL inputs and returns the FULL output.
"""

import numpy as np

N_EXP, E_EXP, FIN, FOUT = 100000, 1_600_000, 128, 128
NCORES = 8
P = 128
GT = 8        # tiles per gather group (fast path)
WINDOW = 32768  # dma_gather int16 index reach


def _numpy_reference(x, edge_row, edge_col, edge_val, weight, bias):
    deg = np.zeros(x.shape[0], np.float64)
    np.add.at(deg, edge_row, edge_val.astype(np.float64))
    dinv = 1.0 / np.sqrt(deg)
    support = np.zeros((x.shape[0], x.shape[1]), np.float64)
    np.add.at(support, edge_row, edge_val[:, None] * x[edge_col].astype(np.float64))
    return (support * dinv[:, None] @ weight + bias).astype(x.dtype)


_BUILD_CACHE = {}




def _cells_for(K, n_src):
    """Fixed position-cuts of the K sorted chunks into cells + index windows.

    Returns (cb, w0): cb[i] = chunks in cell i, w0[i] = source-row base of
    cell i's gather window. Cell i covers sorted positions
    [128*sum(cb[:i]), ...) whose source values concentrate around the
    corresponding quantiles; windows add >5-sigma margin. None if the
    shape can't be handled (caller falls back).
    """
    if n_src <= WINDOW:
        return [K], [0]
    ncell = 5
    if K < ncell:
        return None
    base, rem = divmod(K, ncell)
    cb = [base + (1 if i < rem else 0) for i in range(ncell)]
    E_t = 128 * K
    bounds = np.cumsum([0] + cb) * 128 / E_t  # quantiles of cell starts
    w0 = []
    for i in range(ncell):
        qlo = bounds[i]
        sig = np.sqrt(max(qlo * (1 - qlo), 1e-9) / E_t) * n_src
        lo = max(0, int(qlo * n_src - 5.5 * sig))
        lo = min(lo, n_src - WINDOW)
        w0.append(lo)
        # verify the cell's upper quantile fits the window
        qhi = bounds[i + 1]
        sigh = np.sqrt(max(qhi * (1 - qhi), 1e-9) / E_t) * n_src
        hi = min(n_src - 1, int(qhi * n_src + 5.5 * sigh))
        if hi - lo > WINDOW - 1:
            return None
    return cb, w0


def _build_fast(NG, K, cb, apply_bias, n_src, timing=False, reps=1, mode="full"):
    """Fast-path SPMD bass kernel. NG groups of GT tiles, K slots/dest-row,
    cb = chunks per cell (len 5 or 1, sum == K). mode: "full" | "gather"
    (skip compute, diagnostic) | "compute" (skip gathers, diagnostic)."""
    import concourse.bacc as bacc
    import concourse.mybir as mybir
    import concourse.tile as tile

    key = ("fast", NG, K, tuple(cb), apply_bias, n_src, timing, reps, mode,
           _build_fast._sp)
    if key in _BUILD_CACHE:
        return _BUILD_CACHE[key]

    ncell = len(cb)
    w0s = _cells_for(K, n_src)[1]
    SCH = GT * K                    # chunks per group in msgs
    IDXF = SCH * 128 // 16          # idx free columns per group
    boff = np.cumsum([0] + cb)      # cell chunk offset within a tile
    goff = np.cumsum([0] + [GT * c for c in cb])  # bank block offset in msgs

    nc = bacc.Bacc("TRN2", target_bir_lowering=False, debug=False, num_devices=NCORES,
                   num_swdge_queues=4)
    xb = nc.declare_dram_parameter("xb", [n_src, FIN], mybir.dt.bfloat16, isOutput=False)
    idx = nc.declare_dram_parameter("idx", [NG, P, IDXF], mybir.dt.int16, isOutput=False)
    dstg = nc.declare_dram_parameter("dstg", [NG, P, SCH], mybir.dt.bfloat16, isOutput=False)
    iota = nc.declare_dram_parameter("iota", [P, max(cb) * P], mybir.dt.bfloat16, isOutput=False)
    w = nc.declare_dram_parameter("w", [FIN, FOUT], mybir.dt.bfloat16, isOutput=False)
    if apply_bias:
        biasb = nc.declare_dram_parameter("biasb", [P, FOUT], mybir.dt.float32, isOutput=False)
    if timing:
        out = nc.dram_tensor("scratch", [NG, P, GT * FOUT], mybir.dt.float32)
        tiny = nc.declare_dram_parameter("tiny", [P, 1], mybir.dt.float32, isOutput=True)
    else:
        out = nc.declare_dram_parameter("out", [NG, P, GT * FOUT], mybir.dt.float32, isOutput=True)

    with tile.TileContext(nc) as tc:
        with (
            tc.tile_pool(name="const", bufs=1) as const_pool,
            tc.tile_pool(name="msgs", bufs=3) as msgs_pool,
            tc.tile_pool(name="idxp", bufs=3) as idx_pool,
            tc.tile_pool(name="dstp", bufs=3) as dst_pool,
            tc.tile_pool(name="ohp", bufs=3) as oh_pool,
            tc.tile_pool(name="sup", bufs=3) as sup_pool,
            tc.tile_pool(name="outp", bufs=3) as out_pool,
            tc.tile_pool(name="ps", bufs=4, space="PSUM") as psum_pool,
            tc.tile_pool(name="ps2", bufs=4, space="PSUM") as psum2_pool,
        ):
            iota_sb = const_pool.tile([P, max(cb) * P], mybir.dt.bfloat16)
            nc.sync.dma_start(out=iota_sb[:], in_=iota[:])
            w_sb = const_pool.tile([FIN, FOUT], mybir.dt.bfloat16)
            nc.sync.dma_start(out=w_sb[:], in_=w[:])
            if apply_bias:
                bias_sb = const_pool.tile([P, FOUT], mybir.dt.float32)
                nc.sync.dma_start(out=bias_sb[:], in_=biasb[:])

            msgs_const = None
            for _ in range(reps):
                for g in range(NG):
                    idx_t = idx_pool.tile([P, IDXF], mybir.dt.int16)
                    nc.sync.dma_start(out=idx_t[:], in_=idx[g])
                    dst_t = dst_pool.tile([P, SCH], mybir.dt.bfloat16)
                    nc.sync.dma_start(out=dst_t[:], in_=dstg[g])

                    # Gathers split per (cell, tile-pair): <=1024 idxs each
                    # (Q7 scratch cap), rotated over the 4 SWDGE queues so
                    # descriptor generation and ring drains overlap — single
                    # queue serializes at ~7ns/row, 4 queues reach ~1.2.
                    if mode == "compute":
                        if msgs_const is None:
                            msgs_const = const_pool.tile([P, SCH, FIN], mybir.dt.bfloat16)
                            nc.vector.memset(msgs_const[:, :, :], 0.25)
                        msgs = msgs_const
                    else:
                        msgs = msgs_pool.tile([P, SCH, FIN], mybir.dt.bfloat16)
                    qn = 0
                    for b in range(ncell if mode != "compute" else 0):
                        w0 = w0s[b]
                        src_view = xb[w0 : min(n_src, w0 + WINDOW)]
                        for p2 in range(0, GT, 2):
                            c0 = goff[b] + p2 * cb[b]
                            nch = 2 * cb[b]
                            nidx = nch * 128
                            nc.gpsimd.dma_gather(
                                msgs[:, c0 : c0 + nch, :],
                                src_view,
                                idx_t[:, c0 * 8 : (c0 + nch) * 8],
                                nidx,
                                nidx,
                                FIN,
                                queue_num=qn % 4,
                                single_packet=_build_fast._sp,
                            )
                            qn += 1

                    out_sb = out_pool.tile([P, GT * FOUT], mybir.dt.float32)
                    if mode == "gather":
                        nc.scalar.copy(out_sb[:, 0:2], msgs[:, 0, 0:2])
                        nc.scalar.dma_start(out=out[g][:, 0:2], in_=out_sb[:, 0:2])
                        continue
                    for u in range(GT):
                        oh_t = oh_pool.tile([P, K * P], mybir.dt.bfloat16)
                        for b in range(ncell):
                            nch = cb[b]
                            c0 = goff[b] + u * nch
                            nc.vector.tensor_tensor(
                                out=oh_t[:, boff[b] * P : boff[b + 1] * P],
                                in0=dst_t[:, c0 : c0 + nch].to_broadcast([P, nch, P]),
                                in1=iota_sb[:, : nch * P],
                                op=mybir.AluOpType.is_equal,
                            )
                        supT_ps = psum_pool.tile([FIN, P], mybir.dt.float32, space="PSUM")
                        s = 0
                        for b in range(ncell):
                            for ci in range(cb[b]):
                                nc.tensor.matmul(
                                    out=supT_ps[:],
                                    lhsT=msgs[:, goff[b] + u * cb[b] + ci, :],
                                    rhs=oh_t[:, (boff[b] + ci) * P : (boff[b] + ci + 1) * P],
                                    start=(s == 0),
                                    stop=(s == K - 1),
                                )
                                s += 1
                        supT_sb = sup_pool.tile([FIN, P], mybir.dt.bfloat16)
                        nc.scalar.copy(supT_sb[:], supT_ps[:])

                        out_ps = psum2_pool.tile([P, FOUT], mybir.dt.float32, space="PSUM")
                        nc.tensor.matmul(
                            out=out_ps[:], lhsT=supT_sb[:], rhs=w_sb[:],
                            start=True, stop=True,
                        )
                        if apply_bias:
                            nc.vector.tensor_tensor(
                                out=out_sb[:, u * FOUT : (u + 1) * FOUT],
                                in0=out_ps[:], in1=bias_sb[:],
                                op=mybir.AluOpType.add,
                            )
                        else:
                            nc.vector.tensor_copy(out_sb[:, u * FOUT : (u + 1) * FOUT], out_ps[:])
                    nc.sync.dma_start(out=out[g], in_=out_sb[:])
            if timing:
                nc.sync.dma_start(out=tiny[:], in_=out_sb[:, 0:1])
    nc.compile()
    _BUILD_CACHE[key] = nc
    return nc


def _prepare_fast(x, src_pad, K, N, n_src, weight, bias):
    """Sorted-cell grids for the fast path. Returns (meta, in_maps) or None."""
    import ml_dtypes

    cells = _cells_for(K, n_src)
    if cells is None:
        return None
    cb, w0 = cells
    ncell = len(cb)

    tile_quant = P * GT
    R_core = -(-N // (NCORES * tile_quant)) * tile_quant
    T = R_core // P
    NG = T // GT
    N_pad = R_core * NCORES
    E_t = P * K

    if N_pad > src_pad.shape[0]:
        npad = N_pad - src_pad.shape[0]
        # pseudo-uniform sources for discarded pad dests keep the sorted
        # cells' source ranges quantile-shaped (a block of zeros would
        # push real edges outside their cell windows)
        fake = ((np.arange(npad * K, dtype=np.int64) * 2654435761) % n_src).astype(np.int32)
        src_pad = np.concatenate([src_pad, fake.reshape(npad, K)])

    xb = np.ascontiguousarray(x.astype(ml_dtypes.bfloat16))
    w_eff = np.ascontiguousarray(
        (weight.astype(np.float32) / np.sqrt(K)).astype(ml_dtypes.bfloat16)
    )
    cbm = max(cb)
    iota = np.tile(np.arange(P, dtype=np.float32), cbm)[None, :].repeat(P, 0)
    iota = iota.astype(ml_dtypes.bfloat16)

    apply_bias = bool(np.any(bias != 0.0))
    biasb = np.tile(bias.astype(np.float32)[None, :], (P, 1))

    pos_cuts = np.cumsum([0] + cb) * 128        # edge-position cuts per tile
    cell_of_pos = np.repeat(np.arange(ncell), np.array(cb) * 128)  # [E_t]
    w0_of_pos = np.asarray(w0, np.int64)[cell_of_pos]              # [E_t]

    SCH = GT * K
    in_maps = []
    for c in range(NCORES):
        sl = slice(c * R_core, (c + 1) * R_core)
        srcs = src_pad[sl].reshape(T, E_t)       # [t, d*K+s] dest-major
        dsts = np.repeat(np.arange(P, dtype=np.int64), K)[None, :].repeat(T, 0)
        order = np.argsort(srcs, axis=1, kind="stable")
        ssrc = np.take_along_axis(srcs, order, 1).astype(np.int64)
        sdst = np.take_along_axis(dsts, order, 1)

        loc = ssrc - w0_of_pos[None, :]
        # only discarded pad-dest tiles may violate; verify real rows
        if loc.min() < 0 or loc.max() > WINDOW - 1:
            bad = np.where((loc < 0) | (loc > WINDOW - 1))
            first_bad_row = bad[0].min() * P
            if c * R_core + first_bad_row < N:
                return None  # a real dest would be corrupted: fall back
            loc = np.clip(loc, 0, WINDOW - 1)
        loc16 = loc.astype(np.int16)

        # idx: per (g, cell): cells' [GT, 128*cb] blocks, u-major, wrapped
        # in 16 partitions and replicated x8 -> [NG, 128, IDXF]
        idx_blocks = []
        for b in range(ncell):
            blk = loc16[:, pos_cuts[b] : pos_cuts[b + 1]]      # [T, 128*cb]
            blk = blk.reshape(NG, GT * cb[b] * 128)            # u-major concat
            wrapped = blk.reshape(NG, -1, 16).transpose(0, 2, 1)  # [NG,16,F]
            idx_blocks.append(np.tile(wrapped, (1, 8, 1)))     # [NG,128,F]
        idxg = np.ascontiguousarray(np.concatenate(idx_blocks, axis=2))

        # dst grid: dstg[g, p, chunk(b,u,ci)] = sdst[t, pos_cuts[b]+ci*128+p]
        dst_blocks = []
        for b in range(ncell):
            blk = sdst[:, pos_cuts[b] : pos_cuts[b + 1]]       # [T, cb*128]
            blk = blk.reshape(NG, GT, cb[b], P)                # [g, u, ci, p]
            dst_blocks.append(blk.transpose(0, 3, 1, 2).reshape(NG, P, GT * cb[b]))
        dstgrid = np.concatenate(dst_blocks, axis=2).astype(np.float32)
        dstgrid = np.ascontiguousarray(dstgrid.astype(ml_dtypes.bfloat16))

        m = {"xb": xb, "idx": idxg, "dstg": dstgrid, "iota": iota, "w": w_eff}
        if apply_bias:
            m["biasb"] = biasb
        in_maps.append(m)

    meta = dict(T=T, NG=NG, K=K, cb=cb, w0=w0, fast=True, apply_bias=apply_bias,
                N=N, R_core=R_core, n_src=n_src)
    return meta, in_maps


def _build_general(T, K, apply_val, apply_bias, n_src, timing=False, reps=1):
    """General-path SPMD bass kernel (non-uniform degrees / edge values).
    T dest tiles of 128 rows, K slots/row. Per-slot indirect gathers."""
    import concourse.bacc as bacc
    import concourse.bass as bass
    import concourse.mybir as mybir
    import concourse.tile as tile

    key = ("gen", T, K, apply_val, apply_bias, n_src, timing, reps)
    if key in _BUILD_CACHE:
        return _BUILD_CACHE[key]

    nc = bacc.Bacc("TRN2", target_bir_lowering=False, debug=False, num_devices=NCORES)
    x = nc.declare_dram_parameter("x", [n_src, FIN], mybir.dt.float32, isOutput=False)
    idx = nc.declare_dram_parameter("idx", [T, P, K], mybir.dt.int32, isOutput=False)
    vgrid = nc.declare_dram_parameter("vgrid", [T, P, K], mybir.dt.float32, isOutput=False)
    onehots = nc.declare_dram_parameter("onehots", [P, K * P], mybir.dt.float32, isOutput=False)
    w = nc.declare_dram_parameter("w", [FIN, FOUT], mybir.dt.float32, isOutput=False)
    if apply_val:
        vbatch = nc.declare_dram_parameter("vbatch", [T, P, K], mybir.dt.float32, isOutput=False)
    if apply_bias:
        biasb = nc.declare_dram_parameter("biasb", [P, FOUT], mybir.dt.float32, isOutput=False)
    if timing:
        out = nc.dram_tensor("scratch", [T, P, FOUT], mybir.dt.float32)
        tiny = nc.declare_dram_parameter("tiny", [P, 1], mybir.dt.float32, isOutput=True)
    else:
        out = nc.declare_dram_parameter("out", [T, P, FOUT], mybir.dt.float32, isOutput=True)

    with tile.TileContext(nc) as tc:
        with (
            tc.tile_pool(name="const", bufs=1) as const_pool,
            tc.tile_pool(name="msgs", bufs=3) as msgs_pool,
            tc.tile_pool(name="idxp", bufs=3) as idx_pool,
            tc.tile_pool(name="vgp", bufs=3) as vg_pool,
            tc.tile_pool(name="sup", bufs=2) as sup_pool,
            tc.tile_pool(name="outp", bufs=3) as out_pool,
            tc.tile_pool(name="deg", bufs=2) as deg_pool,
            tc.tile_pool(name="ps", bufs=2, space="PSUM") as psum_pool,
            tc.tile_pool(name="ps2", bufs=2, space="PSUM") as psum2_pool,
        ):
            oh_sb = const_pool.tile([P, K * P], mybir.dt.float32)
            nc.sync.dma_start(out=oh_sb[:], in_=onehots[:])
            w_sb = const_pool.tile([FIN, FOUT], mybir.dt.float32)
            nc.sync.dma_start(out=w_sb[:], in_=w[:])
            if apply_bias:
                bias_sb = const_pool.tile([P, FOUT], mybir.dt.float32)
                nc.sync.dma_start(out=bias_sb[:], in_=biasb[:])

            for _ in range(reps):
                for t in range(T):
                    idx_t = idx_pool.tile([P, K], mybir.dt.int32)
                    nc.sync.dma_start(out=idx_t[:], in_=idx[t])
                    vg_t = vg_pool.tile([P, K], mybir.dt.float32)
                    nc.sync.dma_start(out=vg_t[:], in_=vgrid[t])
                    if apply_val:
                        vb_t = vg_pool.tile([P, K], mybir.dt.float32)
                        nc.sync.dma_start(out=vb_t[:], in_=vbatch[t])

                    msgs = msgs_pool.tile([P, K, FIN], mybir.dt.float32)
                    for s in range(K):
                        nc.gpsimd.indirect_dma_start(
                            out=msgs[:, s, :],
                            out_offset=None,
                            in_=x[:],
                            in_offset=bass.IndirectOffsetOnAxis(
                                ap=idx_t[:, s : s + 1], axis=0
                            ),
                        )
                    if apply_val:
                        for s in range(K):
                            nc.vector.tensor_scalar_mul(
                                msgs[:, s, :], msgs[:, s, :], vb_t[:, s : s + 1]
                            )

                    supT_ps = psum_pool.tile([FIN, P], mybir.dt.float32, space="PSUM")
                    for s in range(K):
                        nc.tensor.matmul(
                            out=supT_ps[:],
                            lhsT=msgs[:, s, :],
                            rhs=oh_sb[:, s * P : (s + 1) * P],
                            start=(s == 0),
                            stop=(s == K - 1),
                        )
                    supT_sb = sup_pool.tile([FIN, P], mybir.dt.float32)
                    nc.vector.tensor_copy(supT_sb[:], supT_ps[:])

                    deg_t = deg_pool.tile([P, 1], mybir.dt.float32)
                    nc.vector.tensor_reduce(
                        out=deg_t[:], in_=vg_t[:],
                        axis=mybir.AxisListType.X, op=mybir.AluOpType.add,
                    )
                    dsq = deg_pool.tile([P, 1], mybir.dt.float32)
                    nc.scalar.sqrt(dsq[:], deg_t[:])
                    dinv = deg_pool.tile([P, 1], mybir.dt.float32)
                    nc.vector.reciprocal(dinv[:], dsq[:])

                    out_ps = psum2_pool.tile([P, FOUT], mybir.dt.float32, space="PSUM")
                    nc.tensor.matmul(
                        out=out_ps[:], lhsT=supT_sb[:], rhs=w_sb[:],
                        start=True, stop=True,
                    )
                    out_sb = out_pool.tile([P, FOUT], mybir.dt.float32)
                    nc.vector.tensor_scalar_mul(out_sb[:], out_ps[:], dinv[:, 0:1])
                    if apply_bias:
                        nc.vector.tensor_tensor(
                            out=out_sb[:], in0=out_sb[:], in1=bias_sb[:],
                            op=mybir.AluOpType.add,
                        )
                    nc.scalar.dma_start(out=out[t], in_=out_sb[:])
            if timing:
                nc.sync.dma_start(out=tiny[:], in_=out_sb[:, 0:1])
    nc.compile()
    _BUILD_CACHE[key] = nc
    return nc


def _prepare(x, edge_row, edge_col, edge_val, weight, bias):
    """Host-side bucketing/sharding. Returns (meta, in_maps)."""
    N = x.shape[0]
    E = edge_row.shape[0]
    n_src = x.shape[0]

    order = np.argsort(edge_row, kind="stable")
    row_s = edge_row[order]
    col_s = edge_col[order]
    val_s = edge_val[order]

    counts = np.bincount(edge_row, minlength=N)
    max_deg = int(counts.max()) if E else 1
    uniform = bool((counts == max_deg).all())
    ones = bool(np.all(edge_val == 1.0))

    K = 1
    while K < max_deg:
        K *= 2
    if K > 128:
        return None  # numpy fallback
    fast = uniform and ones and max_deg == K

    if fast:
        src_pad = col_s.reshape(N, K).astype(np.int32)
        prep = _prepare_fast(x, src_pad, K, N, n_src, weight, bias)
        if prep is not None:
            return prep
        # else fall through to general path

    R_core = -(-N // (NCORES * P)) * P
    T = R_core // P
    N_pad = R_core * NCORES

    if fast:
        val_pad = val_s.reshape(N, K).astype(np.float32)
    else:
        src_pad = np.zeros((N, K), np.int32)
        val_pad = np.zeros((N, K), np.float32)
        pos = np.arange(E) - np.repeat(np.cumsum(counts) - counts, counts)
        src_pad[row_s, pos] = col_s
        val_pad[row_s, pos] = val_s
    if N_pad > N:
        src_pad = np.concatenate([src_pad[:N], np.zeros((N_pad - N, K), np.int32)])
        val_pad = np.concatenate([val_pad, np.zeros((N_pad - N, K), np.float32)])

    e_ar = np.arange(P)
    oh = np.zeros((P, K * P), np.float32)
    for s in range(K):
        oh[e_ar, s * P + s * (P // K) + e_ar // K] = 1.0

    apply_bias = bool(np.any(bias != 0.0))
    biasb = np.tile(bias.astype(np.float32)[None, :], (P, 1))

    x32 = np.ascontiguousarray(x.astype(np.float32))
    w32 = np.ascontiguousarray(weight.astype(np.float32))

    in_maps = []
    for c in range(NCORES):
        sl = slice(c * R_core, (c + 1) * R_core)
        src_c = src_pad[sl]
        val_c = val_pad[sl]
        seq_src = src_c.reshape(T, P * K)
        seq_val = val_c.reshape(T, P * K)
        jj = np.arange(P)[:, None] + np.arange(K)[None, :] * P
        idx_g = seq_src[:, jj.reshape(-1)].reshape(T, P, K).astype(np.int32)
        vb_g = seq_val[:, jj.reshape(-1)].reshape(T, P, K).astype(np.float32)
        vg_g = val_c.reshape(T, P, K).astype(np.float32)
        m = {
            "x": x32,
            "idx": np.ascontiguousarray(idx_g),
            "vgrid": np.ascontiguousarray(vg_g),
            "onehots": oh,
            "w": w32,
        }
        if not ones:
            m["vbatch"] = np.ascontiguousarray(vb_g)
        if apply_bias:
            m["biasb"] = biasb
        in_maps.append(m)
    meta = dict(T=T, K=K, fast=False, apply_val=not ones, apply_bias=apply_bias,
                N=N, R_core=R_core, n_src=n_src)
    return meta, in_maps


def _gather_fast(res, meta):
    NG, R_core = meta["NG"], meta["R_core"]
    outs = []
    for c in range(NCORES):
        o = res.results[c]["out"].reshape(NG, P, GT, FOUT)
        outs.append(np.ascontiguousarray(o.transpose(0, 2, 1, 3)).reshape(R_core, FOUT))
    return np.concatenate(outs, axis=0)[: meta["N"]]


_build_fast._sp = True


def kernel(x, edge_row, edge_col, edge_val, weight, bias):
    x = np.asarray(x)
    edge_row = np.asarray(edge_row)
    edge_col = np.asarray(edge_col)
    edge_val = np.asarray(edge_val)
    weight = np.asarray(weight)
    bias = np.asarray(bias)

    prep = _prepare(x, edge_row, edge_col, edge_val, weight, bias)
    if prep is None:
        return _numpy_reference(x, edge_row, edge_col, edge_val, weight, bias)
    meta, in_maps = prep

    from concourse.bass_utils import run_bass_kernel_spmd

    if meta["fast"]:
        nc = _build_fast(meta["NG"], meta["K"], meta["cb"], meta["apply_bias"],
                         meta["n_src"])
        res = run_bass_kernel_spmd(nc, in_maps, list(range(NCORES)))
        full = _gather_fast(res, meta)
    else:
        nc = _build_general(meta["T"], meta["K"], meta["apply_val"],
                            meta["apply_bias"], meta["n_src"])
        res = run_bass_kernel_spmd(nc, in_maps, list(range(NCORES)))
        outs = [res.results[c]["out"].reshape(meta["R_core"], FOUT)
                for c in range(NCORES)]
        full = np.concatenate(outs, axis=0)[: meta["N"]]
    return full.astype(x.dtype)


# revision 26
# speedup vs baseline: 1.6199x; 1.6199x over previous
"""GCNConv forward on 8 Trainium2 NeuronCores.

out = D^{-1/2} @ A @ x @ W + bias,  A sparse (edge list), D = row-degree.

Strategy: shard destination rows across the 8 cores; edge lists bucketed
by destination row on the host; x replicated per core so each core
gathers its source rows from local HBM; the segment-sum rides the
TensorEngine as matmuls against one-hot matrices; the 128x128 weight is
applied as a second matmul.

Fast path (uniform degree K, unit edge values): x pre-cast to bf16 on
the host; per 128-dest tile the K*128 edges are SORTED BY SOURCE and cut
at fixed positions into 5 cells, each spanning < 32768 source rows so a
multi-index dma_gather (InstDMAGatherAnt, int16 indices against a
per-cell base window) fetches 128*CB rows per instruction — one DMA
descriptor per edge, amortizing the ~1us/instruction SWDGE fixed
overhead that bounds the naive per-slot indirect-DMA version. Because
the sort permutes edges, dest ids travel as data and the one-hot
aggregation matrices are built on-device with batched is_equal against
an iota constant. The uniform D^{-1/2} = 1/sqrt(K) is folded into W on
the host; matmuls run in bf16 with fp32 PSUM accumulation.

kernel() accepts the FULL inputs and returns the FULL output.
"""

import numpy as np

N_EXP, E_EXP, FIN, FOUT = 100000, 1_600_000, 128, 128
NCORES = 8
P = 128
GT = 8        # tiles per gather group (fast path)
WINDOW = 32768  # dma_gather int16 index reach


def _numpy_reference(x, edge_row, edge_col, edge_val, weight, bias):
    deg = np.zeros(x.shape[0], np.float64)
    np.add.at(deg, edge_row, edge_val.astype(np.float64))
    dinv = 1.0 / np.sqrt(deg)
    support = np.zeros((x.shape[0], x.shape[1]), np.float64)
    np.add.at(support, edge_row, edge_val[:, None] * x[edge_col].astype(np.float64))
    return (support * dinv[:, None] @ weight + bias).astype(x.dtype)


_BUILD_CACHE = {}




def _cells_for(K, n_src):
    """Fixed position-cuts of the K sorted chunks into cells + index windows.

    Returns (cb, w0): cb[i] = chunks in cell i, w0[i] = source-row base of
    cell i's gather window. Cell i covers sorted positions
    [128*sum(cb[:i]), ...) whose source values concentrate around the
    corresponding quantiles; windows add >5-sigma margin. None if the
    shape can't be handled (caller falls back).
    """
    if n_src <= WINDOW:
        return [K], [0]
    ncell = 5
    if K < ncell:
        return None
    base, rem = divmod(K, ncell)
    cb = [base + (1 if i < rem else 0) for i in range(ncell)]
    E_t = 128 * K
    bounds = np.cumsum([0] + cb) * 128 / E_t  # quantiles of cell starts
    w0 = []
    for i in range(ncell):
        qlo = bounds[i]
        sig = np.sqrt(max(qlo * (1 - qlo), 1e-9) / E_t) * n_src
        lo = max(0, int(qlo * n_src - 5.5 * sig))
        lo = min(lo, n_src - WINDOW)
        w0.append(lo)
        # verify the cell's upper quantile fits the window
        qhi = bounds[i + 1]
        sigh = np.sqrt(max(qhi * (1 - qhi), 1e-9) / E_t) * n_src
        hi = min(n_src - 1, int(qhi * n_src + 5.5 * sigh))
        if hi - lo > WINDOW - 1:
            return None
    return cb, w0


def _build_fast(NG, K, cb, apply_bias, n_src, timing=False, reps=1, mode="full"):
    """Fast-path SPMD bass kernel. NG groups of GT tiles, K slots/dest-row,
    cb = chunks per cell (len 5 or 1, sum == K). mode: "full" | "gather"
    (skip compute, diagnostic) | "compute" (skip gathers, diagnostic)."""
    import concourse.bacc as bacc
    import concourse.mybir as mybir
    import concourse.tile as tile

    key = ("fast", NG, K, tuple(cb), apply_bias, n_src, timing, reps, mode,
           _build_fast._sp)
    if key in _BUILD_CACHE:
        return _BUILD_CACHE[key]

    ncell = len(cb)
    w0s = _cells_for(K, n_src)[1]
    SCH = GT * K                    # chunks per group in msgs
    IDXF = SCH * 128 // 16          # idx free columns per group
    boff = np.cumsum([0] + cb)      # cell chunk offset within a tile
    goff = np.cumsum([0] + [GT * c for c in cb])  # bank block offset in msgs

    nc = bacc.Bacc("TRN2", target_bir_lowering=False, debug=False, num_devices=NCORES,
                   num_swdge_queues=4)
    xb = nc.declare_dram_parameter("xb", [n_src, FIN], mybir.dt.bfloat16, isOutput=False)
    idx = nc.declare_dram_parameter("idx", [NG, P, IDXF], mybir.dt.int16, isOutput=False)
    dstg = nc.declare_dram_parameter("dstg", [NG, P, SCH], mybir.dt.bfloat16, isOutput=False)
    iota = nc.declare_dram_parameter("iota", [P, max(cb) * P], mybir.dt.bfloat16, isOutput=False)
    w = nc.declare_dram_parameter("w", [FIN, FOUT], mybir.dt.bfloat16, isOutput=False)
    if apply_bias:
        biasb = nc.declare_dram_parameter("biasb", [P, FOUT], mybir.dt.float32, isOutput=False)
    if timing:
        out = nc.dram_tensor("scratch", [NG, P, GT * FOUT], mybir.dt.float32)
        tiny = nc.declare_dram_parameter("tiny", [P, 1], mybir.dt.float32, isOutput=True)
    else:
        out = nc.declare_dram_parameter("out", [NG, P, GT * FOUT], mybir.dt.float32, isOutput=True)

    with tile.TileContext(nc) as tc:
        with (
            tc.tile_pool(name="const", bufs=1) as const_pool,
            tc.tile_pool(name="msgs", bufs=4) as msgs_pool,
            tc.tile_pool(name="idxp", bufs=3) as idx_pool,
            tc.tile_pool(name="dstp", bufs=3) as dst_pool,
            tc.tile_pool(name="ohp", bufs=3) as oh_pool,
            tc.tile_pool(name="sup", bufs=3) as sup_pool,
            tc.tile_pool(name="outp", bufs=3) as out_pool,
            tc.tile_pool(name="ps", bufs=4, space="PSUM") as psum_pool,
            tc.tile_pool(name="ps2", bufs=4, space="PSUM") as psum2_pool,
        ):
            iota_sb = const_pool.tile([P, max(cb) * P], mybir.dt.bfloat16)
            nc.sync.dma_start(out=iota_sb[:], in_=iota[:])
            w_sb = const_pool.tile([FIN, FOUT], mybir.dt.bfloat16)
            nc.sync.dma_start(out=w_sb[:], in_=w[:])
            if apply_bias:
                bias_sb = const_pool.tile([P, FOUT], mybir.dt.float32)
                nc.sync.dma_start(out=bias_sb[:], in_=biasb[:])

            msgs_const = None
            for _ in range(reps):
                for g in range(NG):
                    idx_t = idx_pool.tile([P, IDXF], mybir.dt.int16)
                    nc.sync.dma_start(out=idx_t[:], in_=idx[g])
                    dst_t = dst_pool.tile([P, SCH], mybir.dt.bfloat16)
                    nc.sync.dma_start(out=dst_t[:], in_=dstg[g])

                    # Gathers split per (cell, tile-pair): <=1024 idxs each
                    # (Q7 scratch cap), rotated over the 4 SWDGE queues so
                    # descriptor generation and ring drains overlap — single
                    # queue serializes at ~7ns/row, 4 queues reach ~1.2.
                    if mode == "compute":
                        if msgs_const is None:
                            msgs_const = const_pool.tile([P, SCH, FIN], mybir.dt.bfloat16)
                            nc.vector.memset(msgs_const[:, :, :], 0.25)
                        msgs = msgs_const
                    else:
                        msgs = msgs_pool.tile([P, SCH, FIN], mybir.dt.bfloat16)
                    qn = 0
                    for b in range(ncell if mode != "compute" else 0):
                        w0 = w0s[b]
                        src_view = xb[w0 : min(n_src, w0 + WINDOW)]
                        for p2 in range(0, GT, 2):
                            c0 = goff[b] + p2 * cb[b]
                            nch = 2 * cb[b]
                            nidx = nch * 128
                            nc.gpsimd.dma_gather(
                                msgs[:, c0 : c0 + nch, :],
                                src_view,
                                idx_t[:, c0 * 8 : (c0 + nch) * 8],
                                nidx,
                                nidx,
                                FIN,
                                queue_num=qn % 4,
                                single_packet=_build_fast._sp,
                            )
                            qn += 1

                    out_sb = out_pool.tile([P, GT * FOUT], mybir.dt.float32)
                    if mode == "gather":
                        nc.scalar.copy(out_sb[:, 0:2], msgs[:, 0, 0:2])
                        nc.scalar.dma_start(out=out[g][:, 0:2], in_=out_sb[:, 0:2])
                        continue
                    for u in range(GT):
                        oh_t = oh_pool.tile([P, K * P], mybir.dt.bfloat16)
                        for b in range(ncell):
                            nch = cb[b]
                            c0 = goff[b] + u * nch
                            nc.vector.tensor_tensor(
                                out=oh_t[:, boff[b] * P : boff[b + 1] * P],
                                in0=dst_t[:, c0 : c0 + nch].to_broadcast([P, nch, P]),
                                in1=iota_sb[:, : nch * P],
                                op=mybir.AluOpType.is_equal,
                            )
                        supT_ps = psum_pool.tile([FIN, P], mybir.dt.float32, space="PSUM")
                        s = 0
                        for b in range(ncell):
                            for ci in range(cb[b]):
                                nc.tensor.matmul(
                                    out=supT_ps[:],
                                    lhsT=msgs[:, goff[b] + u * cb[b] + ci, :],
                                    rhs=oh_t[:, (boff[b] + ci) * P : (boff[b] + ci + 1) * P],
                                    start=(s == 0),
                                    stop=(s == K - 1),
                                )
                                s += 1
                        supT_sb = sup_pool.tile([FIN, P], mybir.dt.bfloat16)
                        nc.scalar.copy(supT_sb[:], supT_ps[:])

                        out_ps = psum2_pool.tile([P, FOUT], mybir.dt.float32, space="PSUM")
                        nc.tensor.matmul(
                            out=out_ps[:], lhsT=supT_sb[:], rhs=w_sb[:],
                            start=True, stop=True,
                        )
                        if apply_bias:
                            nc.vector.tensor_tensor(
                                out=out_sb[:, u * FOUT : (u + 1) * FOUT],
                                in0=out_ps[:], in1=bias_sb[:],
                                op=mybir.AluOpType.add,
                            )
                        else:
                            nc.vector.tensor_copy(out_sb[:, u * FOUT : (u + 1) * FOUT], out_ps[:])
                    nc.sync.dma_start(out=out[g], in_=out_sb[:])
            if timing:
                nc.sync.dma_start(out=tiny[:], in_=out_sb[:, 0:1])
    nc.compile()
    _BUILD_CACHE[key] = nc
    return nc


def _prepare_fast(x, src_pad, K, N, n_src, weight, bias):
    """Sorted-cell grids for the fast path. Returns (meta, in_maps) or None."""
    import ml_dtypes

    cells = _cells_for(K, n_src)
    if cells is None:
        return None
    cb, w0 = cells
    ncell = len(cb)

    tile_quant = P * GT
    R_core = -(-N // (NCORES * tile_quant)) * tile_quant
    T = R_core // P
    NG = T // GT
    N_pad = R_core * NCORES
    E_t = P * K

    if N_pad > src_pad.shape[0]:
        npad = N_pad - src_pad.shape[0]
        # pseudo-uniform sources for discarded pad dests keep the sorted
        # cells' source ranges quantile-shaped (a block of zeros would
        # push real edges outside their cell windows)
        fake = ((np.arange(npad * K, dtype=np.int64) * 2654435761) % n_src).astype(np.int32)
        src_pad = np.concatenate([src_pad, fake.reshape(npad, K)])

    xb = np.ascontiguousarray(x.astype(ml_dtypes.bfloat16))
    w_eff = np.ascontiguousarray(
        (weight.astype(np.float32) / np.sqrt(K)).astype(ml_dtypes.bfloat16)
    )
    cbm = max(cb)
    iota = np.tile(np.arange(P, dtype=np.float32), cbm)[None, :].repeat(P, 0)
    iota = iota.astype(ml_dtypes.bfloat16)

    apply_bias = bool(np.any(bias != 0.0))
    biasb = np.tile(bias.astype(np.float32)[None, :], (P, 1))

    pos_cuts = np.cumsum([0] + cb) * 128        # edge-position cuts per tile
    cell_of_pos = np.repeat(np.arange(ncell), np.array(cb) * 128)  # [E_t]
    w0_of_pos = np.asarray(w0, np.int64)[cell_of_pos]              # [E_t]

    SCH = GT * K
    in_maps = []
    for c in range(NCORES):
        sl = slice(c * R_core, (c + 1) * R_core)
        srcs = src_pad[sl].reshape(T, E_t)       # [t, d*K+s] dest-major
        dsts = np.repeat(np.arange(P, dtype=np.int64), K)[None, :].repeat(T, 0)
        order = np.argsort(srcs, axis=1, kind="stable")
        ssrc = np.take_along_axis(srcs, order, 1).astype(np.int64)
        sdst = np.take_along_axis(dsts, order, 1)

        loc = ssrc - w0_of_pos[None, :]
        # only discarded pad-dest tiles may violate; verify real rows
        if loc.min() < 0 or loc.max() > WINDOW - 1:
            bad = np.where((loc < 0) | (loc > WINDOW - 1))
            first_bad_row = bad[0].min() * P
            if c * R_core + first_bad_row < N:
                return None  # a real dest would be corrupted: fall back
            loc = np.clip(loc, 0, WINDOW - 1)
        loc16 = loc.astype(np.int16)

        # idx: per (g, cell): cells' [GT, 128*cb] blocks, u-major, wrapped
        # in 16 partitions and replicated x8 -> [NG, 128, IDXF]
        idx_blocks = []
        for b in range(ncell):
            blk = loc16[:, pos_cuts[b] : pos_cuts[b + 1]]      # [T, 128*cb]
            blk = blk.reshape(NG, GT * cb[b] * 128)            # u-major concat
            wrapped = blk.reshape(NG, -1, 16).transpose(0, 2, 1)  # [NG,16,F]
            idx_blocks.append(np.tile(wrapped, (1, 8, 1)))     # [NG,128,F]
        idxg = np.ascontiguousarray(np.concatenate(idx_blocks, axis=2))

        # dst grid: dstg[g, p, chunk(b,u,ci)] = sdst[t, pos_cuts[b]+ci*128+p]
        dst_blocks = []
        for b in range(ncell):
            blk = sdst[:, pos_cuts[b] : pos_cuts[b + 1]]       # [T, cb*128]
            blk = blk.reshape(NG, GT, cb[b], P)                # [g, u, ci, p]
            dst_blocks.append(blk.transpose(0, 3, 1, 2).reshape(NG, P, GT * cb[b]))
        dstgrid = np.concatenate(dst_blocks, axis=2).astype(np.float32)
        dstgrid = np.ascontiguousarray(dstgrid.astype(ml_dtypes.bfloat16))

        m = {"xb": xb, "idx": idxg, "dstg": dstgrid, "iota": iota, "w": w_eff}
        if apply_bias:
            m["biasb"] = biasb
        in_maps.append(m)

    meta = dict(T=T, NG=NG, K=K, cb=cb, w0=w0, fast=True, apply_bias=apply_bias,
                N=N, R_core=R_core, n_src=n_src)
    return meta, in_maps


def _build_general(T, K, apply_val, apply_bias, n_src, timing=False, reps=1):
    """General-path SPMD bass kernel (non-uniform degrees / edge values).
    T dest tiles of 128 rows, K slots/row. Per-slot indirect gathers."""
    import concourse.bacc as bacc
    import concourse.bass as bass
    import concourse.mybir as mybir
    import concourse.tile as tile

    key = ("gen", T, K, apply_val, apply_bias, n_src, timing, reps)
    if key in _BUILD_CACHE:
        return _BUILD_CACHE[key]

    nc = bacc.Bacc("TRN2", target_bir_lowering=False, debug=False, num_devices=NCORES)
    x = nc.declare_dram_parameter("x", [n_src, FIN], mybir.dt.float32, isOutput=False)
    idx = nc.declare_dram_parameter("idx", [T, P, K], mybir.dt.int32, isOutput=False)
    vgrid = nc.declare_dram_parameter("vgrid", [T, P, K], mybir.dt.float32, isOutput=False)
    onehots = nc.declare_dram_parameter("onehots", [P, K * P], mybir.dt.float32, isOutput=False)
    w = nc.declare_dram_parameter("w", [FIN, FOUT], mybir.dt.float32, isOutput=False)
    if apply_val:
        vbatch = nc.declare_dram_parameter("vbatch", [T, P, K], mybir.dt.float32, isOutput=False)
    if apply_bias:
        biasb = nc.declare_dram_parameter("biasb", [P, FOUT], mybir.dt.float32, isOutput=False)
    if timing:
        out = nc.dram_tensor("scratch", [T, P, FOUT], mybir.dt.float32)
        tiny = nc.declare_dram_parameter("tiny", [P, 1], mybir.dt.float32, isOutput=True)
    else:
        out = nc.declare_dram_parameter("out", [T, P, FOUT], mybir.dt.float32, isOutput=True)

    with tile.TileContext(nc) as tc:
        with (
            tc.tile_pool(name="const", bufs=1) as const_pool,
            tc.tile_pool(name="msgs", bufs=4) as msgs_pool,
            tc.tile_pool(name="idxp", bufs=3) as idx_pool,
            tc.tile_pool(name="vgp", bufs=3) as vg_pool,
            tc.tile_pool(name="sup", bufs=2) as sup_pool,
            tc.tile_pool(name="outp", bufs=3) as out_pool,
            tc.tile_pool(name="deg", bufs=2) as deg_pool,
            tc.tile_pool(name="ps", bufs=2, space="PSUM") as psum_pool,
            tc.tile_pool(name="ps2", bufs=2, space="PSUM") as psum2_pool,
        ):
            oh_sb = const_pool.tile([P, K * P], mybir.dt.float32)
            nc.sync.dma_start(out=oh_sb[:], in_=onehots[:])
            w_sb = const_pool.tile([FIN, FOUT], mybir.dt.float32)
            nc.sync.dma_start(out=w_sb[:], in_=w[:])
            if apply_bias:
                bias_sb = const_pool.tile([P, FOUT], mybir.dt.float32)
                nc.sync.dma_start(out=bias_sb[:], in_=biasb[:])

            for _ in range(reps):
                for t in range(T):
                    idx_t = idx_pool.tile([P, K], mybir.dt.int32)
                    nc.sync.dma_start(out=idx_t[:], in_=idx[t])
                    vg_t = vg_pool.tile([P, K], mybir.dt.float32)
                    nc.sync.dma_start(out=vg_t[:], in_=vgrid[t])
                    if apply_val:
                        vb_t = vg_pool.tile([P, K], mybir.dt.float32)
                        nc.sync.dma_start(out=vb_t[:], in_=vbatch[t])

                    msgs = msgs_pool.tile([P, K, FIN], mybir.dt.float32)
                    for s in range(K):
                        nc.gpsimd.indirect_dma_start(
                            out=msgs[:, s, :],
                            out_offset=None,
                            in_=x[:],
                            in_offset=bass.IndirectOffsetOnAxis(
                                ap=idx_t[:, s : s + 1], axis=0
                            ),
                        )
                    if apply_val:
                        for s in range(K):
                            nc.vector.tensor_scalar_mul(
                                msgs[:, s, :], msgs[:, s, :], vb_t[:, s : s + 1]
                            )

                    supT_ps = psum_pool.tile([FIN, P], mybir.dt.float32, space="PSUM")
                    for s in range(K):
                        nc.tensor.matmul(
                            out=supT_ps[:],
                            lhsT=msgs[:, s, :],
                            rhs=oh_sb[:, s * P : (s + 1) * P],
                            start=(s == 0),
                            stop=(s == K - 1),
                        )
                    supT_sb = sup_pool.tile([FIN, P], mybir.dt.float32)
                    nc.vector.tensor_copy(supT_sb[:], supT_ps[:])

                    deg_t = deg_pool.tile([P, 1], mybir.dt.float32)
                    nc.vector.tensor_reduce(
                        out=deg_t[:], in_=vg_t[:],
                        axis=mybir.AxisListType.X, op=mybir.AluOpType.add,
                    )
                    dsq = deg_pool.tile([P, 1], mybir.dt.float32)
                    nc.scalar.sqrt(dsq[:], deg_t[:])
                    dinv = deg_pool.tile([P, 1], mybir.dt.float32)
                    nc.vector.reciprocal(dinv[:], dsq[:])

                    out_ps = psum2_pool.tile([P, FOUT], mybir.dt.float32, space="PSUM")
                    nc.tensor.matmul(
                        out=out_ps[:], lhsT=supT_sb[:], rhs=w_sb[:],
                        start=True, stop=True,
                    )
                    out_sb = out_pool.tile([P, FOUT], mybir.dt.float32)
                    nc.vector.tensor_scalar_mul(out_sb[:], out_ps[:], dinv[:, 0:1])
                    if apply_bias:
                        nc.vector.tensor_tensor(
                            out=out_sb[:], in0=out_sb[:], in1=bias_sb[:],
                            op=mybir.AluOpType.add,
                        )
                    nc.scalar.dma_start(out=out[t], in_=out_sb[:])
            if timing:
                nc.sync.dma_start(out=tiny[:], in_=out_sb[:, 0:1])
    nc.compile()
    _BUILD_CACHE[key] = nc
    return nc


def _prepare(x, edge_row, edge_col, edge_val, weight, bias):
    """Host-side bucketing/sharding. Returns (meta, in_maps)."""
    N = x.shape[0]
    E = edge_row.shape[0]
    n_src = x.shape[0]

    order = np.argsort(edge_row, kind="stable")
    row_s = edge_row[order]
    col_s = edge_col[order]
    val_s = edge_val[order]

    counts = np.bincount(edge_row, minlength=N)
    max_deg = int(counts.max()) if E else 1
    uniform = bool((counts == max_deg).all())
    ones = bool(np.all(edge_val == 1.0))

    K = 1
    while K < max_deg:
        K *= 2
    if K > 128:
        return None  # numpy fallback
    fast = uniform and ones and max_deg == K

    if fast:
        src_pad = col_s.reshape(N, K).astype(np.int32)
        prep = _prepare_fast(x, src_pad, K, N, n_src, weight, bias)
        if prep is not None:
            return prep
        # else fall through to general path

    R_core = -(-N // (NCORES * P)) * P
    T = R_core // P
    N_pad = R_core * NCORES

    if fast:
        val_pad = val_s.reshape(N, K).astype(np.float32)
    else:
        src_pad = np.zeros((N, K), np.int32)
        val_pad = np.zeros((N, K), np.float32)
        pos = np.arange(E) - np.repeat(np.cumsum(counts) - counts, counts)
        src_pad[row_s, pos] = col_s
        val_pad[row_s, pos] = val_s
    if N_pad > N:
        src_pad = np.concatenate([src_pad[:N], np.zeros((N_pad - N, K), np.int32)])
        val_pad = np.concatenate([val_pad, np.zeros((N_pad - N, K), np.float32)])

    e_ar = np.arange(P)
    oh = np.zeros((P, K * P), np.float32)
    for s in range(K):
        oh[e_ar, s * P + s * (P // K) + e_ar // K] = 1.0

    apply_bias = bool(np.any(bias != 0.0))
    biasb = np.tile(bias.astype(np.float32)[None, :], (P, 1))

    x32 = np.ascontiguousarray(x.astype(np.float32))
    w32 = np.ascontiguousarray(weight.astype(np.float32))

    in_maps = []
    for c in range(NCORES):
        sl = slice(c * R_core, (c + 1) * R_core)
        src_c = src_pad[sl]
        val_c = val_pad[sl]
        seq_src = src_c.reshape(T, P * K)
        seq_val = val_c.reshape(T, P * K)
        jj = np.arange(P)[:, None] + np.arange(K)[None, :] * P
        idx_g = seq_src[:, jj.reshape(-1)].reshape(T, P, K).astype(np.int32)
        vb_g = seq_val[:, jj.reshape(-1)].reshape(T, P, K).astype(np.float32)
        vg_g = val_c.reshape(T, P, K).astype(np.float32)
        m = {
            "x": x32,
            "idx": np.ascontiguousarray(idx_g),
            "vgrid": np.ascontiguousarray(vg_g),
            "onehots": oh,
            "w": w32,
        }
        if not ones:
            m["vbatch"] = np.ascontiguousarray(vb_g)
        if apply_bias:
            m["biasb"] = biasb
        in_maps.append(m)
    meta = dict(T=T, K=K, fast=False, apply_val=not ones, apply_bias=apply_bias,
                N=N, R_core=R_core, n_src=n_src)
    return meta, in_maps


def _gather_fast(res, meta):
    NG, R_core = meta["NG"], meta["R_core"]
    outs = []
    for c in range(NCORES):
        o = res.results[c]["out"].reshape(NG, P, GT, FOUT)
        outs.append(np.ascontiguousarray(o.transpose(0, 2, 1, 3)).reshape(R_core, FOUT))
    return np.concatenate(outs, axis=0)[: meta["N"]]


_build_fast._sp = True


def kernel(x, edge_row, edge_col, edge_val, weight, bias):
    x = np.asarray(x)
    edge_row = np.asarray(edge_row)
    edge_col = np.asarray(edge_col)
    edge_val = np.asarray(edge_val)
    weight = np.asarray(weight)
    bias = np.asarray(bias)

    prep = _prepare(x, edge_row, edge_col, edge_val, weight, bias)
    if prep is None:
        return _numpy_reference(x, edge_row, edge_col, edge_val, weight, bias)
    meta, in_maps = prep

    from concourse.bass_utils import run_bass_kernel_spmd

    if meta["fast"]:
        nc = _build_fast(meta["NG"], meta["K"], meta["cb"], meta["apply_bias"],
                         meta["n_src"])
        res = run_bass_kernel_spmd(nc, in_maps, list(range(NCORES)))
        full = _gather_fast(res, meta)
    else:
        nc = _build_general(meta["T"], meta["K"], meta["apply_val"],
                            meta["apply_bias"], meta["n_src"])
        res = run_bass_kernel_spmd(nc, in_maps, list(range(NCORES)))
        outs = [res.results[c]["out"].reshape(meta["R_core"], FOUT)
                for c in range(NCORES)]
        full = np.concatenate(outs, axis=0)[: meta["N"]]
    return full.astype(x.dtype)
